# revision 64
# baseline (speedup 1.0000x reference)
"""Trainium2 Bass kernel for a 12-head causal attention block with RoPE.

Module: qkv = x @ w_qkv.T; rope(q), rope(k); causal softmax attention;
out @ w_proj.T + b_proj.  Shapes: x [4, 2048, 768], 12 heads, Dh=64.

Sharding (8 cores): core = 2*b + hg handles batch b and head-group hg
(6 heads), processed as 3 head-pairs.  Each core returns 3 pair-partial
projection outputs y^T [768, 2048] (bf16); the host sums the 6 partials
per batch and adds b_proj.

On-core dataflow (channel-major; bf16 operands, fp32 psum):
  - x^T resident in SBUF; QKV projections as bf16 matmuls (separate
    LDWEIGHTS pipelines ahead of the streams, FWL engages).
  - RoPE with a parity-split head channel order so the pair rotation is
    a 16-lane swap inside each 32-partition quadrant (stream_shuffle),
    then two multiplies and an add against host-built cos/sin tables.
  - Scores computed transposed (S^T[j,i] = K @ Q^T) with two heads
    row-packed in the PE array (K=64 each, tile_position (0,0)/(64,0)),
    both written into one 2-bank psum tile so a single ACT instruction
    exponentiates the pair.  Softmax skips the row-max (scores here are
    O(1)); P^T = exp(scale*S^T) in bf16, masked by a 0/1 lower-triangle
    multiply on diagonal blocks only; strictly-upper blocks are never
    computed (block-causal skip).
  - PV is computed flipped: stationary = P^T[j, i-sub-block(128)],
    moving = [V_h(64) | 1] (65 cols, FWL hides the weight loads), so a
    whole (i,j) block costs 65 PE cycles instead of ~512; the appended
    ones column emits the softmax denominator as output column 64.
    PSUM allows only one open accumulation group per 2KB zero region,
    so the per-i-sub-block accumulation passes run serially per bank
    (the two heads' banks interleave) after the I-block's exps.
  - Normalization is a reciprocal + one stride-0-broadcast multiply in
    the [i, d] orientation; a PE transpose (via identity) restores
    [dh, tok] for the projection.
  - Projection runs per 512-token block right after its normalization,
    overlapping the next attention block; the next pair's QKV+RoPE is
    software-pipelined into the current pair's attention loop (fed in
    consumption order: the last pair runs descending-I so the kernel
    tail is the small I=0 block, and receives all K blocks first).
  - A ~5us dummy-matmul warm-up flips the HAM clock gate to 2.4GHz
    while the batched input DMAs (issued in consumption order) stream.
"""

import sys

sys.path.insert(0, "/opt/trn_rl_repo")

import numpy as np
import ml_dtypes

BF = ml_dtypes.bfloat16

B, N, C, H, Dh = 4, 2048, 768, 12, 64
NCORES = 8
NPAIRS = 3  # head pairs per core
NI = 4      # 512-token i-super blocks
NJ = 16     # 128-token j blocks
SCALE = Dh ** -0.5

_compiled = None


def _perm64():
    """sbuf row p_l (0..63) -> original head-channel d (parity-split order)."""
    perm = np.empty(64, dtype=np.int64)
    for p in range(64):
        q_l, m = p // 32, p % 32
        r = q_l * 16 + (m % 16)
        perm[p] = 2 * r + (0 if m < 16 else 1)
    return perm


def _build_program():
    import concourse.bass as bass
    import concourse.mybir as mybir
    import concourse.tile as tile
    from concourse import bacc

    F32, F32R = mybir.dt.float32, mybir.dt.float32r
    BF16 = mybir.dt.bfloat16
    AF = mybir.ActivationFunctionType
    OP = mybir.AluOpType

    nc = bacc.Bacc(None, target_bir_lowering=False)

    xT = nc.dram_tensor("xT", [C, N], BF16, kind="ExternalInput")
    wqkT = nc.dram_tensor("wqkT", [NPAIRS, C, 256], BF16, kind="ExternalInput")
    wvT = nc.dram_tensor("wvT", [C, 384], BF16, kind="ExternalInput")
    wpT = nc.dram_tensor("wpT", [NPAIRS, 128, C], BF16, kind="ExternalInput")
    c2T = nc.dram_tensor("c2T", [128, N], BF16, kind="ExternalInput")
    s2T = nc.dram_tensor("s2T", [128, N], BF16, kind="ExternalInput")
    tri01 = nc.dram_tensor("tri01", [128, 128], BF16, kind="ExternalInput")
    identT = nc.dram_tensor("ident", [128, 128], BF16, kind="ExternalInput")
    ypart = nc.dram_tensor("ypart", [NPAIRS, C, N], BF16, kind="ExternalOutput")

    swap_mask = list(range(16, 32)) + list(range(0, 16))

    with tile.TileContext(nc) as tc:
        with (
            tc.tile_pool(name="res", bufs=1) as res,
            tc.tile_pool(name="mm", bufs=2, space="PSUM") as mmps,
            tc.tile_pool(name="st", bufs=2, space="PSUM") as stps,
            tc.tile_pool(name="ot", bufs=1, space="PSUM") as otps,
        ):
            # ---- PE warm-up: ~4.5us of dummy matmuls with no DMA deps, so
            # the HAM clock gate flips to 2.4GHz while the input DMAs stream
            # in.  The vv fill is emitted immediately after so the DVE queue
            # is past it before the V-block copies arrive.
            wsb = res.tile([128, 512], BF16, tag="wsb")
            nc.vector.memset(wsb[:], 0.25)
            # V layout per j-block, per pair: [V_A(64)|1 | V_B(64)|1]; the
            # ones column appended to each head's V makes the flipped PV
            # matmul emit the softmax denominator as output column 64
            vv = res.tile([128, NJ, 390], BF16, tag="vv")
            vvt = vv[:].tensor
            nc.vector.memset(vv[:], 1.0)
            wps = mmps.tile([128, 512], F32, tag="mm", name="warmps")
            for r in range(16):
                nc.tensor.matmul(wps[:], wsb[:, 0:128], wsb[:],
                                 start=(r == 0), stop=(r == 15))

            # ---- resident loads (token-chunked x so early V/QK blocks can
            # start as soon as their token range lands) ----
            xt = res.tile([128, 6, N], BF16, tag="xt")
            wv = res.tile([128, 6, 384], BF16, tag="wv")
            c2 = res.tile([128, N], BF16, tag="c2")
            s2 = res.tile([128, N], BF16, tag="s2")
            tri = res.tile([128, 128], BF16, tag="tri")
            idm = res.tile([128, 128], BF16, tag="idm")
            # batched DMAs (one instruction each): dma_start issue is
            # serialized on the Sync engine at ~0.7us per instruction, and
            # transfers complete in issue order at ~270GB/s aggregate --
            # issue in consumption order (V tb0-3, qk tb0, masks, rope
            # tables, then the remaining token blocks)
            xTv = xT[:].rearrange("(c p) n -> p c n", p=128)
            wvv = wvT[:].rearrange("(c p) n -> p c n", p=128)
            nc.sync.dma_start(xt[:, :, 0:512], xTv[:, :, 0:512])
            nc.sync.dma_start(wv[:], wvv)
            # prewarm the exp table load off the critical path; the copy on
            # the (otherwise idle) Scalar engine also gives the warm-up psum
            # tile the reader the BIR verifier requires
            warm = res.tile([1, 8], F32, tag="warm")
            nc.scalar.copy(warm[:], wps[0:1, 0:8])
            nc.scalar.activation(warm[:], warm[:], AF.Exp, scale=1.0)

            def emit_v_block(tb):
                pv = mmps.tile([128, 384], F32, tag="mm", name=f"pv{tb}")
                for ct in range(6):
                    nc.tensor.matmul(
                        pv[:], xt[:, ct, tb * 128:(tb + 1) * 128], wv[:, ct, :],
                        start=(ct == 0), stop=(ct == 5),
                    )
                dst = bass.AP(
                    tensor=vvt, offset=tb * 390,
                    ap=[[NJ * 390, 128], [130, NPAIRS], [65, 2], [1, 64]],
                )
                src = pv[:].rearrange("p (a s d) -> p a s d", a=NPAIRS, s=2, d=64)
                # Scalar engine: it is idle during the preamble, while the
                # DVE is the critical path for the pair-0 rope chains
                nc.scalar.copy(dst, src)

            # ---- head pairs ----
            with (
                tc.tile_pool(name="wq", bufs=2) as wpool,
                tc.tile_pool(name="qk", bufs=3) as qkpool,
                tc.tile_pool(name="pt", bufs=18) as ptpool,
                tc.tile_pool(name="tmp", bufs=6) as tmppool,
                tc.tile_pool(name="onrm", bufs=2) as onrmpool,
            ):
                def emit_w_dma(p):
                    wqk = wpool.tile([128, 6, 256], BF16, tag="wqk")
                    nc.sync.dma_start(
                        wqk[:], wqkT[p].rearrange("(c p) n -> p c n", p=128))
                    wpj = wpool.tile([128, C], BF16, tag="wpj")
                    nc.sync.dma_start(wpj[:], wpT[p, :, :])
                    return wqk, wpj

                def emit_qk_block(state, idx, add_on_gpsimd=False):
                    wqk, qt, kt = state["wqk"], state["qt"], state["kt"]
                    sec, tb = idx % 2, idx // 2
                    dest = qt if sec == 0 else kt
                    pqk = mmps.tile([128, 512], F32, tag="mm")
                    tok = slice(tb * 512, (tb + 1) * 512)
                    for ct in range(6):
                        nc.tensor.matmul(
                            pqk[:], wqk[:, ct, sec * 128:(sec + 1) * 128],
                            xt[:, ct, tok],
                            start=(ct == 0), stop=(ct == 5),
                        )
                    # rope: out = psum*C2 + shuffle(psum)*S2
                    tsh = tmppool.tile([128, 512], F32, tag="tsh")
                    tms = tmppool.tile([128, 512], F32, tag="tms")
                    tmc = tmppool.tile([128, 512], F32, tag="tmc")
                    nc.vector.stream_shuffle(tsh[:], pqk[:], swap_mask)
                    nc.gpsimd.tensor_tensor(tms[:], tsh[:], s2[:, tok], OP.mult)
                    nc.vector.tensor_tensor(tmc[:], pqk[:], c2[:, tok], OP.mult)
                    # the add has all-SBUF operands so it can go on GpSimd;
                    # only worth it in the DVE-bound preamble
                    eng = nc.gpsimd if add_on_gpsimd else nc.vector
                    eng.tensor_tensor(dest[:, tb, :], tmc[:], tms[:], OP.add)

                def new_pair_state(p):
                    wqk, wpj = emit_w_dma(p)
                    return {
                        "wqk": wqk, "wpj": wpj,
                        "qt": qkpool.tile([128, NI, 512], BF16, tag="qt", name=f"qt{p}"),
                        "kt": qkpool.tile([128, NI, 512], BF16, tag="kt", name=f"kt{p}"),
                    }

                state = new_pair_state(0)
                # small mask/rope tables next (needed by pair-0's first
                # diagonal blocks and rope chains), then the remaining
                # x token blocks
                nc.sync.dma_start(tri[:], tri01[:])
                nc.sync.dma_start(idm[:], identT[:])
                nc.sync.dma_start(c2[:], c2T[:])
                nc.sync.dma_start(s2[:], s2T[:])
                for tb4 in range(1, 4):
                    nc.sync.dma_start(xt[:, :, tb4 * 512:(tb4 + 1) * 512],
                                      xTv[:, :, tb4 * 512:(tb4 + 1) * 512])

                # interleave V blocks with pair-0 QK+rope so the PE stays busy
                # while the rope chains run on DVE/GpSimd; later V blocks are
                # deprioritized (only needed at PV time) so pair-0's first
                # scores win the PE as soon as their rope chains land
                for tb4 in range(4):
                    for tb in range(4 * tb4, 4 * tb4 + 4):
                        if tb < 12:
                            emit_v_block(tb)
                        else:
                            # needed only by I=3's last PV pass -- let pair-0
                            # scores win the PE first
                            with tc.high_priority(offset=-150):
                                emit_v_block(tb)
                    emit_qk_block(state, 2 * tb4)
                    emit_qk_block(state, 2 * tb4 + 1)

                for p in range(NPAIRS):
                    wpj = state["wpj"]
                    qt, kt = state["qt"], state["kt"]
                    next_state = new_pair_state(p + 1) if p + 1 < NPAIRS else None

                    outNT = onrmpool.tile([128, NI, 512], BF16, tag="outNT")
                    # last pair runs largest-I first so the kernel ends on
                    # the small I=0 block instead of the 16-block I=3 chain;
                    # earlier pairs ascend (pair 0 is paced by the x DMA)
                    iorder = (3, 2, 1, 0) if p == NPAIRS - 1 else (0, 1, 2, 3)
                    for iloop, I in enumerate(iorder):
                        # flipped PV: stationary = P^T[j, i-sub-128-block],
                        # moving = [V_h | 1] (65 cols) -> out[i, d] accumulates
                        # per i-sub-block; col 64 is the softmax denominator.
                        # oP cols: h*512 + isub*65 + d (each head in its own
                        # psum bank so no matmul write crosses a bank)
                        oP = otps.tile([128, 1024], F32, tag="oP")
                        oPt = oP[:].tensor
                        njb = 4 * I + 4
                        pabs = []
                        pms = []
                        for jb in range(njb):
                            t = jb - 4 * I
                            # scores stream from sc0 (moving dim kept >= 256
                            # for full rate); exp/mask cover only the valid
                            # columns from c0
                            c0 = 0 if t < 1 else 128 * t
                            sc0 = min(c0, 256)
                            cs = slice(sc0, 512)
                            jb4 = jb // 4
                            jbs = slice((jb % 4) * 128, (jb % 4) * 128 + 128)
                            sAB = stps.tile([128, 1024], F32, tag="sAB")
                            nc.tensor.matmul(
                                sAB[:, cs], kt[0:64, jb4, jbs], qt[0:64, I, cs],
                                start=True, stop=True, tile_position=(0, 0),
                            )
                            nc.tensor.matmul(
                                sAB[:, 512 + sc0:1024],
                                kt[64:128, jb4, jbs], qt[64:128, I, cs],
                                start=True, stop=True, tile_position=(64, 0),
                            )
                            pAB = ptpool.tile([128, 1024], BF16, tag="pAB")
                            pabs.append(pAB)
                            sv = sAB[:].rearrange("p (h c) -> p h c", h=2)
                            pv_ = pAB[:].rearrange("p (h c) -> p h c", h=2)
                            with tc.high_priority(offset=40):
                                nc.scalar.activation(
                                    pv_[:, :, c0:512], sv[:, :, c0:512],
                                    AF.Exp, scale=SCALE)
                            if t >= 0:
                                # masked diag product into a fresh tile (one
                                # strided TT for both heads; not in-place so
                                # the DVE fast path can engage); PV's diag
                                # pass reads pm instead of pAB
                                pm = tmppool.tile([128, 256], BF16, tag="pm",
                                                  name=f"pm{p}_{I}_{t}")
                                nc.vector.tensor_tensor(
                                    pm[:].rearrange("q (h c) -> q h c", h=2),
                                    bass.AP(tensor=pAB[:].tensor, offset=c0,
                                            ap=[[1024, 128], [512, 2],
                                                [1, 128]]),
                                    bass.AP(tensor=tri[:].tensor, offset=0,
                                            ap=[[128, 128], [0, 2], [1, 128]]),
                                    OP.mult)
                                pms.append(pm)
                        # accumulation passes: one open psum group per bank
                        # (2KB zero region) at a time -- isub passes serialize
                        # within each head's bank, the two heads interleave
                        for isub in range(4):
                            for jb in range(4 * I + isub + 1):
                                diag = (jb - 4 * I == isub)
                                for h in range(2):
                                    if diag:
                                        statnry = pms[isub][:, h * 128:
                                                            (h + 1) * 128]
                                    else:
                                        statnry = pabs[jb][:, h * 512 +
                                                           isub * 128:
                                                           h * 512 +
                                                           isub * 128 + 128]
                                    nc.tensor.matmul(
                                        bass.AP(tensor=oPt,
                                                offset=h * 512 + isub * 65,
                                                ap=[[1024, 128], [1, 65]]),
                                        statnry,
                                        vv[:, jb, p * 130 + h * 65:
                                           p * 130 + (h + 1) * 65],
                                        start=(jb == 0),
                                        stop=(jb == 4 * I + isub),
                                    )
                        # normalize: out[i, d] * (1/L[i]) with L broadcast
                        # along d via a stride-0 AP, then DMA-transpose the
                        # bf16 result back to [dh, tok] for the projection
                        rL = onrmpool.tile([128, 8], F32, tag="rL")
                        oN = onrmpool.tile([128, 512], BF16, tag="oN")
                        oNt = oN[:].tensor
                        with tc.high_priority():
                            nc.vector.reciprocal_approx_fast(
                                rL[:],
                                bass.AP(tensor=oPt, offset=64,
                                        ap=[[1024, 128], [512, 2], [65, 4]]))
                            nc.vector.tensor_tensor(
                                bass.AP(tensor=oNt, offset=0,
                                        ap=[[512, 128], [64, 2], [128, 4],
                                            [1, 64]]),
                                bass.AP(tensor=oPt, offset=0,
                                        ap=[[1024, 128], [512, 2], [65, 4],
                                            [1, 64]]),
                                bass.AP(tensor=rL[:].tensor, offset=0,
                                        ap=[[8, 128], [4, 2], [1, 4],
                                            [0, 64]]),
                                OP.mult)
                        oT = otps.tile([128, 512], BF16, tag="oP",
                                       name=f"oT{p}_{I}")
                        for isub in range(4):
                            nc.tensor.transpose(
                                oT[:, isub * 128:(isub + 1) * 128],
                                oN[:, isub * 128:(isub + 1) * 128], idm[:])
                        nc.vector.tensor_copy(outNT[:, I, :], oT[:])

                        # projection for this I's token block, overlaps with
                        # the next I's attention; ys blocks gather into one
                        # tile, stored as two half DMAs so the first store
                        # overlaps the remaining casts
                        ysg = onrmpool.tile([128, 6, 512], BF16, tag="ysg")
                        ypv = ypart[p].rearrange("(c p) n -> p c n", p=128)
                        for ocb in range(6):
                            py = mmps.tile([128, 512], F32, tag="mm")
                            nc.tensor.matmul(
                                py[:], wpj[:, ocb * 128:(ocb + 1) * 128],
                                outNT[:, I, :],
                                start=True, stop=True,
                            )
                            if ocb == 4:
                                nc.scalar.copy(ysg[:, ocb, :], py[:])
                            else:
                                nc.vector.tensor_copy(ysg[:, ocb, :], py[:])
                            if ocb == 2 or ocb == 5:
                                h3 = slice(ocb - 2, ocb + 1)
                                nc.sync.dma_start(
                                    ypv[:, h3, I * 512:(I + 1) * 512],
                                    ysg[:, h3, :],
                                )

                        # pipeline next pair's qkv+rope into this attention;
                        # deprioritized so ready score/PV matmuls win the
                        # Tensor queue over this filler work.  When the next
                        # pair runs descending-I, feed it all K blocks then
                        # Q3..Q0 (its first block needs qt3 + every kt)
                        if next_state is not None:
                            if p + 1 == NPAIRS - 1:
                                qorder = (1, 3, 5, 7, 6, 4, 2, 0)
                            else:
                                qorder = (0, 1, 2, 3, 4, 5, 6, 7)
                            with tc.high_priority(offset=-400):
                                emit_qk_block(next_state, qorder[2 * iloop])
                                emit_qk_block(next_state, qorder[2 * iloop + 1])

                    if next_state is not None:
                        state = next_state

    nc.compile()
    return nc


def _host_prep(x, freqs_cos, freqs_sin, mask, w_qkv, w_proj):
    """Build per-core input maps."""
    perm = _perm64()

    r_of_p = np.empty(128, dtype=np.int64)
    sign_of_p = np.empty(128, dtype=np.float32)
    for pp in range(128):
        p_l = pp % 64
        q_l, m = p_l // 32, p_l % 32
        r_of_p[pp] = q_l * 16 + (m % 16)
        sign_of_p[pp] = -1.0 if m < 16 else 1.0
    c2T = np.ascontiguousarray(freqs_cos.T[r_of_p, :], dtype=np.float32)
    s2T = np.ascontiguousarray(
        freqs_sin.T[r_of_p, :] * sign_of_p[:, None], dtype=np.float32)

    # 0/1 lower-triangle (transposed causal) tile from the provided mask:
    # valid (j <= i) where mask[0,0,i,j] == 0 -> tri01[j, i] = 1
    m0 = mask[0, 0, :128, :128]
    tri01 = np.ascontiguousarray((m0.T == 0).astype(np.float32))
    tri256 = np.zeros((128, 256), dtype=np.float32)
    tri256[:, 128:] = tri01

    in_maps = []
    for core in range(NCORES):
        b, hg = core // 2, core % 2
        heads = [hg * 6 + i for i in range(6)]
        xTc = np.ascontiguousarray(x[b].T)

        wqkT = np.empty((NPAIRS, C, 256), dtype=np.float32)
        wpTc = np.empty((NPAIRS, 128, C), dtype=np.float32)
        for p in range(NPAIRS):
            for hh in range(2):
                hgl = heads[2 * p + hh]
                rows_q = 0 * C + hgl * 64 + perm
                rows_k = 1 * C + hgl * 64 + perm
                wqkT[p, :, hh * 64:(hh + 1) * 64] = w_qkv[rows_q, :].T
                wqkT[p, :, 128 + hh * 64:128 + (hh + 1) * 64] = w_qkv[rows_k, :].T
                wpTc[p, hh * 64:(hh + 1) * 64, :] = \
                    w_proj[:, hgl * 64:(hgl + 1) * 64].T
        wvTc = np.empty((C, 384), dtype=np.float32)
        for i, hgl in enumerate(heads):
            rows_v = 2 * C + hgl * 64 + np.arange(64)
            wvTc[:, i * 64:(i + 1) * 64] = w_qkv[rows_v, :].T

        in_maps.append({
            "xT": xTc.astype(BF),
            "wqkT": np.ascontiguousarray(wqkT).astype(BF),
            "wvT": wvTc.astype(BF),
            "wpT": np.ascontiguousarray(wpTc).astype(BF),
            "c2T": c2T.astype(BF),
            "s2T": s2T.astype(BF),
            "tri01": tri01.astype(BF),
            "tri256": tri256.astype(BF),
            "ident": np.eye(128, dtype=np.float32).astype(BF),
        })
    return in_maps


def _mask_is_causal(mask):
    m = mask[0, 0]
    if m.shape != (N, N):
        return False
    iu = np.triu_indices(N, k=1)
    il = np.tril_indices(N, k=0)
    return bool(np.all(m[il] == 0.0) and np.all(m[iu] <= -1e8))


def _numpy_reference(x, freqs_cos, freqs_sin, mask, w_qkv, w_proj, b_proj):
    """Exact fallback (never expected: setup_inputs' mask is causal)."""
    Bq, Nq, Cq = x.shape
    qkv = x @ w_qkv.T
    qkv = qkv.reshape(Bq, Nq, 3, H, Dh)
    q, k, v = qkv[:, :, 0], qkv[:, :, 1], qkv[:, :, 2]

    def rope(t):
        tr = t.reshape(Bq, Nq, H, Dh // 2, 2)
        a, b = tr[..., 0], tr[..., 1]
        c = freqs_cos[None, :, None, :]
        s = freqs_sin[None, :, None, :]
        return np.stack([a * c - b * s, a * s + b * c], axis=-1).reshape(t.shape)

    q, k = rope(q), rope(k)
    q = q.transpose(0, 2, 1, 3)
    k = k.transpose(0, 2, 1, 3)
    v = v.transpose(0, 2, 1, 3)
    att = np.einsum('bhqd,bhkd->bhqk', q, k) * SCALE + mask
    att = att - att.max(axis=-1, keepdims=True)
    att = np.exp(att)
    att = att / att.sum(axis=-1, keepdims=True)
    out = np.einsum('bhqk,bhkd->bhqd', att, v)
    out = out.transpose(0, 2, 1, 3).reshape(Bq, Nq, Cq)
    return (out @ w_proj.T + b_proj).astype(np.float32)


def _get_compiled():
    global _compiled
    if _compiled is None:
        _compiled = _build_program()
    return _compiled


def run_device(in_maps, trace=False, trace_kwargs=None):
    from concourse.bass_utils import run_bass_kernel_spmd
    nc = _get_compiled()
    kwargs = {}
    if trace:
        kwargs["trace"] = True
        if trace_kwargs:
            kwargs["trace_kwargs"] = trace_kwargs
    return run_bass_kernel_spmd(nc, in_maps, core_ids=list(range(NCORES)), **kwargs)


def _assemble(results, b_proj):
    y = np.empty((B, N, C), dtype=np.float32)
    for b in range(B):
        acc = results[2 * b]["ypart"].astype(np.float32).sum(axis=0)
        acc += results[2 * b + 1]["ypart"].astype(np.float32).sum(axis=0)
        y[b] = acc.T + b_proj[None, :]
    return y


def kernel(x, freqs_cos, freqs_sin, mask, w_qkv, w_proj, b_proj):
    x = np.asarray(x, dtype=np.float32)
    freqs_cos = np.asarray(freqs_cos, dtype=np.float32)
    freqs_sin = np.asarray(freqs_sin, dtype=np.float32)
    mask = np.asarray(mask, dtype=np.float32)
    w_qkv = np.asarray(w_qkv, dtype=np.float32)
    w_proj = np.asarray(w_proj, dtype=np.float32)
    b_proj = np.asarray(b_proj, dtype=np.float32)

    if x.shape != (B, N, C) or not _mask_is_causal(mask):
        return _numpy_reference(x, freqs_cos, freqs_sin, mask, w_qkv, w_proj, b_proj)

    in_maps = _host_prep(x, freqs_cos, freqs_sin, mask, w_qkv, w_proj)
    res = run_device(in_maps)
    return _assemble(res.results, b_proj)



# revision 65
# speedup vs baseline: 1.2050x; 1.2050x over previous
"""Trainium2 Bass kernel for a 12-head causal attention block with RoPE.

Module: qkv = x @ w_qkv.T; rope(q), rope(k); causal softmax attention;
out @ w_proj.T + b_proj.  Shapes: x [4, 2048, 768], 12 heads, Dh=64.

Sharding (8 cores): core = 2*b + hg handles batch b and head-group hg
(6 heads), processed as 3 head-pairs.  Each core returns 3 pair-partial
projection outputs y^T [768, 2048] (bf16); the host sums the 6 partials
per batch and adds b_proj.

On-core dataflow (channel-major; bf16 operands, fp32 psum):
  - x^T resident in SBUF; QKV projections as bf16 matmuls (separate
    LDWEIGHTS pipelines ahead of the streams, FWL engages).
  - RoPE with a parity-split head channel order so the pair rotation is
    a 16-lane swap inside each 32-partition quadrant (stream_shuffle),
    then two multiplies and an add against host-built cos/sin tables.
  - Scores computed transposed (S^T[j,i] = K @ Q^T) with two heads
    row-packed in the PE array (K=64 each, tile_position (0,0)/(64,0)),
    both written into one 2-bank psum tile so a single ACT instruction
    exponentiates the pair.  Softmax skips the row-max (scores here are
    O(1)); P^T = exp(scale*S^T) in bf16, masked by a 0/1 lower-triangle
    multiply on diagonal blocks only; strictly-upper blocks are never
    computed (block-causal skip).
  - PV is computed flipped: stationary = P^T[j, i-sub-block(128)],
    moving = [V_h(64) | 1] (65 cols, FWL hides the weight loads), so a
    whole (i,j) block costs 65 PE cycles instead of ~512; the appended
    ones column emits the softmax denominator as output column 64.
    PSUM allows only one open accumulation group per 2KB zero region,
    so the per-i-sub-block accumulation passes run serially per bank
    (the two heads' banks interleave) after the I-block's exps.
  - Normalization is a reciprocal + one stride-0-broadcast multiply in
    the [i, d] orientation; a PE transpose (via identity) restores
    [dh, tok] for the projection.
  - Projection runs per 512-token block right after its normalization,
    overlapping the next attention block; the next pair's QKV+RoPE is
    software-pipelined into the current pair's attention loop (fed in
    consumption order: the last pair runs descending-I so the kernel
    tail is the small I=0 block, and receives all K blocks first).
  - A ~5us dummy-matmul warm-up flips the HAM clock gate to 2.4GHz
    while the batched input DMAs (issued in consumption order) stream.
"""

import sys

sys.path.insert(0, "/opt/trn_rl_repo")

import numpy as np
import ml_dtypes

BF = ml_dtypes.bfloat16

B, N, C, H, Dh = 4, 2048, 768, 12, 64
NCORES = 8
NPAIRS = 3  # head pairs per core
NI = 4      # 512-token i-super blocks
NJ = 16     # 128-token j blocks
SCALE = Dh ** -0.5

_compiled = None


def _perm64():
    """sbuf row p_l (0..63) -> original head-channel d (parity-split order)."""
    perm = np.empty(64, dtype=np.int64)
    for p in range(64):
        q_l, m = p // 32, p % 32
        r = q_l * 16 + (m % 16)
        perm[p] = 2 * r + (0 if m < 16 else 1)
    return perm


def _build_program():
    import concourse.bass as bass
    import concourse.mybir as mybir
    import concourse.tile as tile
    from concourse import bacc

    F32, F32R = mybir.dt.float32, mybir.dt.float32r
    BF16 = mybir.dt.bfloat16
    AF = mybir.ActivationFunctionType
    OP = mybir.AluOpType

    nc = bacc.Bacc(None, target_bir_lowering=False)

    xT = nc.dram_tensor("xT", [C, N], BF16, kind="ExternalInput")
    wqkT = nc.dram_tensor("wqkT", [NPAIRS, C, 256], BF16, kind="ExternalInput")
    wvT = nc.dram_tensor("wvT", [C, 384], BF16, kind="ExternalInput")
    wpT = nc.dram_tensor("wpT", [NPAIRS, 128, C], BF16, kind="ExternalInput")
    c2T = nc.dram_tensor("c2T", [128, N], BF16, kind="ExternalInput")
    s2T = nc.dram_tensor("s2T", [128, N], BF16, kind="ExternalInput")
    tri01 = nc.dram_tensor("tri01", [128, 128], BF16, kind="ExternalInput")
    identT = nc.dram_tensor("ident", [128, 128], BF16, kind="ExternalInput")
    ypart = nc.dram_tensor("ypart", [NPAIRS, C, N], BF16, kind="ExternalOutput")

    swap_mask = list(range(16, 32)) + list(range(0, 16))

    with tile.TileContext(nc) as tc:
        with (
            tc.tile_pool(name="res", bufs=1) as res,
            tc.tile_pool(name="mm", bufs=2, space="PSUM") as mmps,
            tc.tile_pool(name="st", bufs=2, space="PSUM") as stps,
            tc.tile_pool(name="ot", bufs=1, space="PSUM") as otps,
        ):
            # ---- PE warm-up: ~4.5us of dummy matmuls with no DMA deps, so
            # the HAM clock gate flips to 2.4GHz while the input DMAs stream
            # in.  The vv fill is emitted immediately after so the DVE queue
            # is past it before the V-block copies arrive.
            wsb = res.tile([128, 512], BF16, tag="wsb")
            nc.vector.memset(wsb[:], 0.25)
            # V layout per j-block, per pair: [V_A(64)|1 | V_B(64)|1]; the
            # ones column appended to each head's V makes the flipped PV
            # matmul emit the softmax denominator as output column 64
            vv = res.tile([128, NJ, 390], BF16, tag="vv")
            vvt = vv[:].tensor
            nc.vector.memset(vv[:], 1.0)
            wps = mmps.tile([128, 512], F32, tag="mm", name="warmps")
            for r in range(16):
                nc.tensor.matmul(wps[:], wsb[:, 0:128], wsb[:],
                                 start=(r == 0), stop=(r == 15))

            # ---- resident loads (token-chunked x so early V/QK blocks can
            # start as soon as their token range lands) ----
            xt = res.tile([128, 6, N], BF16, tag="xt")
            wv = res.tile([128, 6, 384], BF16, tag="wv")
            c2 = res.tile([128, N], BF16, tag="c2")
            s2 = res.tile([128, N], BF16, tag="s2")
            tri = res.tile([128, 128], BF16, tag="tri")
            idm = res.tile([128, 128], BF16, tag="idm")
            # batched DMAs (one instruction each): dma_start issue is
            # serialized on the Sync engine at ~0.7us per instruction, and
            # transfers complete in issue order at ~270GB/s aggregate --
            # issue in consumption order (V tb0-3, qk tb0, masks, rope
            # tables, then the remaining token blocks)
            xTv = xT[:].rearrange("(c p) n -> p c n", p=128)
            wvv = wvT[:].rearrange("(c p) n -> p c n", p=128)
            nc.sync.dma_start(xt[:, :, 0:512], xTv[:, :, 0:512])
            nc.sync.dma_start(wv[:], wvv)
            # prewarm the exp table load off the critical path; the copy on
            # the (otherwise idle) Scalar engine also gives the warm-up psum
            # tile the reader the BIR verifier requires
            warm = res.tile([1, 8], F32, tag="warm")
            nc.scalar.copy(warm[:], wps[0:1, 0:8])
            nc.scalar.activation(warm[:], warm[:], AF.Exp, scale=1.0)

            def emit_v_block(tb):
                pv = mmps.tile([128, 384], F32, tag="mm", name=f"pv{tb}")
                for ct in range(6):
                    nc.tensor.matmul(
                        pv[:], xt[:, ct, tb * 128:(tb + 1) * 128], wv[:, ct, :],
                        start=(ct == 0), stop=(ct == 5),
                    )
                dst = bass.AP(
                    tensor=vvt, offset=tb * 390,
                    ap=[[NJ * 390, 128], [130, NPAIRS], [65, 2], [1, 64]],
                )
                src = pv[:].rearrange("p (a s d) -> p a s d", a=NPAIRS, s=2, d=64)
                # Scalar engine: it is idle during the preamble, while the
                # DVE is the critical path for the pair-0 rope chains
                nc.scalar.copy(dst, src)

            # ---- head pairs ----
            with (
                tc.tile_pool(name="wq", bufs=2) as wpool,
                tc.tile_pool(name="qk", bufs=3) as qkpool,
                tc.tile_pool(name="pt", bufs=18) as ptpool,
                tc.tile_pool(name="tmp", bufs=6) as tmppool,
                tc.tile_pool(name="onrm", bufs=2) as onrmpool,
            ):
                def emit_w_dma(p):
                    wqk = wpool.tile([128, 6, 256], BF16, tag="wqk")
                    nc.sync.dma_start(
                        wqk[:], wqkT[p].rearrange("(c p) n -> p c n", p=128))
                    wpj = wpool.tile([128, C], BF16, tag="wpj")
                    nc.sync.dma_start(wpj[:], wpT[p, :, :])
                    return wqk, wpj

                def emit_qk_block(state, idx, add_on_gpsimd=False):
                    wqk, qt, kt = state["wqk"], state["qt"], state["kt"]
                    sec, tb = idx % 2, idx // 2
                    dest = qt if sec == 0 else kt
                    pqk = mmps.tile([128, 512], F32, tag="mm")
                    tok = slice(tb * 512, (tb + 1) * 512)
                    for ct in range(6):
                        nc.tensor.matmul(
                            pqk[:], wqk[:, ct, sec * 128:(sec + 1) * 128],
                            xt[:, ct, tok],
                            start=(ct == 0), stop=(ct == 5),
                        )
                    # rope: out = psum*C2 + shuffle(psum)*S2
                    tsh = tmppool.tile([128, 512], F32, tag="tsh")
                    tms = tmppool.tile([128, 512], F32, tag="tms")
                    tmc = tmppool.tile([128, 512], F32, tag="tmc")
                    nc.vector.stream_shuffle(tsh[:], pqk[:], swap_mask)
                    nc.gpsimd.tensor_tensor(tms[:], tsh[:], s2[:, tok], OP.mult)
                    nc.vector.tensor_tensor(tmc[:], pqk[:], c2[:, tok], OP.mult)
                    # the add has all-SBUF operands so it can go on GpSimd;
                    # only worth it in the DVE-bound preamble
                    eng = nc.gpsimd if add_on_gpsimd else nc.vector
                    eng.tensor_tensor(dest[:, tb, :], tmc[:], tms[:], OP.add)

                def new_pair_state(p):
                    wqk, wpj = emit_w_dma(p)
                    return {
                        "wqk": wqk, "wpj": wpj,
                        "qt": qkpool.tile([128, NI, 512], BF16, tag="qt", name=f"qt{p}"),
                        "kt": qkpool.tile([128, NI, 512], BF16, tag="kt", name=f"kt{p}"),
                    }

                state = new_pair_state(0)
                # small mask/rope tables next (needed by pair-0's first
                # diagonal blocks and rope chains), then the remaining
                # x token blocks
                nc.sync.dma_start(tri[:], tri01[:])
                nc.sync.dma_start(idm[:], identT[:])
                nc.sync.dma_start(c2[:], c2T[:])
                nc.sync.dma_start(s2[:], s2T[:])
                for tb4 in range(1, 4):
                    nc.sync.dma_start(xt[:, :, tb4 * 512:(tb4 + 1) * 512],
                                      xTv[:, :, tb4 * 512:(tb4 + 1) * 512])

                # interleave V blocks with pair-0 QK+rope so the PE stays busy
                # while the rope chains run on DVE/GpSimd; later V blocks are
                # deprioritized (only needed at PV time) so pair-0's first
                # scores win the PE as soon as their rope chains land
                for tb4 in range(4):
                    for tb in range(4 * tb4, 4 * tb4 + 4):
                        if tb < 12:
                            emit_v_block(tb)
                        else:
                            # needed only by I=3's last PV pass -- let pair-0
                            # scores win the PE first
                            with tc.high_priority(offset=-150):
                                emit_v_block(tb)
                    emit_qk_block(state, 2 * tb4)
                    emit_qk_block(state, 2 * tb4 + 1)

                for p in range(NPAIRS):
                    wpj = state["wpj"]
                    qt, kt = state["qt"], state["kt"]
                    next_state = new_pair_state(p + 1) if p + 1 < NPAIRS else None

                    outNT = onrmpool.tile([128, NI, 512], BF16, tag="outNT")
                    # last pair runs largest-I first so the kernel ends on
                    # the small I=0 block instead of the 16-block I=3 chain;
                    # earlier pairs ascend (pair 0 is paced by the x DMA)
                    iorder = (3, 2, 1, 0) if p == NPAIRS - 1 else (0, 1, 2, 3)
                    for iloop, I in enumerate(iorder):
                        # flipped PV: stationary = P^T[j, i-sub-128-block],
                        # moving = [V_h | 1] (65 cols) -> out[i, d] accumulates
                        # per i-sub-block; col 64 is the softmax denominator.
                        # oP cols: h*512 + isub*65 + d (each head in its own
                        # psum bank so no matmul write crosses a bank)
                        oP = otps.tile([128, 1024], F32, tag="oP")
                        oPt = oP[:].tensor
                        njb = 4 * I + 4
                        pabs = []
                        pms = []
                        for jb in range(njb):
                            t = jb - 4 * I
                            # scores stream from sc0 (moving dim kept >= 256
                            # for full rate); exp/mask cover only the valid
                            # columns from c0
                            c0 = 0 if t < 1 else 128 * t
                            sc0 = min(c0, 256)
                            cs = slice(sc0, 512)
                            jb4 = jb // 4
                            jbs = slice((jb % 4) * 128, (jb % 4) * 128 + 128)
                            sAB = stps.tile([128, 1024], F32, tag="sAB")
                            nc.tensor.matmul(
                                sAB[:, cs], kt[0:64, jb4, jbs], qt[0:64, I, cs],
                                start=True, stop=True, tile_position=(0, 0),
                            )
                            nc.tensor.matmul(
                                sAB[:, 512 + sc0:1024],
                                kt[64:128, jb4, jbs], qt[64:128, I, cs],
                                start=True, stop=True, tile_position=(64, 0),
                            )
                            pAB = ptpool.tile([128, 1024], BF16, tag="pAB")
                            pabs.append(pAB)
                            sv = sAB[:].rearrange("p (h c) -> p h c", h=2)
                            pv_ = pAB[:].rearrange("p (h c) -> p h c", h=2)
                            with tc.high_priority(offset=40):
                                nc.scalar.activation(
                                    pv_[:, :, c0:512], sv[:, :, c0:512],
                                    AF.Exp, scale=SCALE)
                            if t >= 0:
                                dg = slice(c0, c0 + 128)
                                nc.vector.tensor_tensor(
                                    pAB[:, dg], pAB[:, dg], tri[:], OP.mult)
                                dgB = slice(512 + dg.start, 512 + dg.stop)
                                nc.vector.tensor_tensor(
                                    pAB[:, dgB], pAB[:, dgB], tri[:], OP.mult)
                        # accumulation passes: one open psum group per bank
                        # (2KB zero region) at a time -- isub passes serialize
                        # within each head's bank, the two heads interleave
                        for isub in range(4):
                            for jb in range(4 * I + isub + 1):
                                for h in range(2):
                                    nc.tensor.matmul(
                                        bass.AP(tensor=oPt,
                                                offset=h * 512 + isub * 65,
                                                ap=[[1024, 128], [1, 65]]),
                                        pabs[jb][:, h * 512 + isub * 128:
                                                 h * 512 + isub * 128 + 128],
                                        vv[:, jb, p * 130 + h * 65:
                                           p * 130 + (h + 1) * 65],
                                        start=(jb == 0),
                                        stop=(jb == 4 * I + isub),
                                    )
                        # normalize: out[i, d] * (1/L[i]) with L broadcast
                        # along d via a stride-0 AP, then DMA-transpose the
                        # bf16 result back to [dh, tok] for the projection
                        rL = onrmpool.tile([128, 8], F32, tag="rL")
                        oN = onrmpool.tile([128, 512], BF16, tag="oN")
                        oNt = oN[:].tensor
                        with tc.high_priority():
                            nc.vector.reciprocal_approx_fast(
                                rL[:],
                                bass.AP(tensor=oPt, offset=64,
                                        ap=[[1024, 128], [512, 2], [65, 4]]))
                            nc.vector.tensor_tensor(
                                bass.AP(tensor=oNt, offset=0,
                                        ap=[[512, 128], [64, 2], [128, 4],
                                            [1, 64]]),
                                bass.AP(tensor=oPt, offset=0,
                                        ap=[[1024, 128], [512, 2], [65, 4],
                                            [1, 64]]),
                                bass.AP(tensor=rL[:].tensor, offset=0,
                                        ap=[[8, 128], [4, 2], [1, 4],
                                            [0, 64]]),
                                OP.mult)
                        oT = otps.tile([128, 512], BF16, tag="oP",
                                       name=f"oT{p}_{I}")
                        for isub in range(4):
                            nc.tensor.transpose(
                                oT[:, isub * 128:(isub + 1) * 128],
                                oN[:, isub * 128:(isub + 1) * 128], idm[:])
                        nc.vector.tensor_copy(outNT[:, I, :], oT[:])

                        # projection for this I's token block, overlaps with
                        # the next I's attention; ys blocks gather into one
                        # tile, stored as two half DMAs so the first store
                        # overlaps the remaining casts
                        ysg = onrmpool.tile([128, 6, 512], BF16, tag="ysg")
                        ypv = ypart[p].rearrange("(c p) n -> p c n", p=128)
                        for ocb in range(6):
                            py = mmps.tile([128, 512], F32, tag="mm")
                            nc.tensor.matmul(
                                py[:], wpj[:, ocb * 128:(ocb + 1) * 128],
                                outNT[:, I, :],
                                start=True, stop=True,
                            )
                            if ocb == 4:
                                nc.scalar.copy(ysg[:, ocb, :], py[:])
                            else:
                                nc.vector.tensor_copy(ysg[:, ocb, :], py[:])
                            if ocb == 2 or ocb == 5:
                                h3 = slice(ocb - 2, ocb + 1)
                                nc.sync.dma_start(
                                    ypv[:, h3, I * 512:(I + 1) * 512],
                                    ysg[:, h3, :],
                                )

                        # pipeline next pair's qkv+rope into this attention;
                        # deprioritized so ready score/PV matmuls win the
                        # Tensor queue over this filler work.  When the next
                        # pair runs descending-I, feed it all K blocks then
                        # Q3..Q0 (its first block needs qt3 + every kt)
                        if next_state is not None:
                            if p + 1 == NPAIRS - 1:
                                qorder = (1, 3, 5, 7, 6, 4, 2, 0)
                            else:
                                qorder = (0, 1, 2, 3, 4, 5, 6, 7)
                            with tc.high_priority(offset=-400):
                                emit_qk_block(next_state, qorder[2 * iloop])
                                emit_qk_block(next_state, qorder[2 * iloop + 1])

                    if next_state is not None:
                        state = next_state

    nc.compile()
    return nc


def _host_prep(x, freqs_cos, freqs_sin, mask, w_qkv, w_proj):
    """Build per-core input maps."""
    perm = _perm64()

    r_of_p = np.empty(128, dtype=np.int64)
    sign_of_p = np.empty(128, dtype=np.float32)
    for pp in range(128):
        p_l = pp % 64
        q_l, m = p_l // 32, p_l % 32
        r_of_p[pp] = q_l * 16 + (m % 16)
        sign_of_p[pp] = -1.0 if m < 16 else 1.0
    c2T = np.ascontiguousarray(freqs_cos.T[r_of_p, :], dtype=np.float32)
    s2T = np.ascontiguousarray(
        freqs_sin.T[r_of_p, :] * sign_of_p[:, None], dtype=np.float32)

    # 0/1 lower-triangle (transposed causal) tile from the provided mask:
    # valid (j <= i) where mask[0,0,i,j] == 0 -> tri01[j, i] = 1
    m0 = mask[0, 0, :128, :128]
    tri01 = np.ascontiguousarray((m0.T == 0).astype(np.float32))
    tri256 = np.zeros((128, 256), dtype=np.float32)
    tri256[:, 128:] = tri01

    in_maps = []
    for core in range(NCORES):
        b, hg = core // 2, core % 2
        heads = [hg * 6 + i for i in range(6)]
        xTc = np.ascontiguousarray(x[b].T)

        wqkT = np.empty((NPAIRS, C, 256), dtype=np.float32)
        wpTc = np.empty((NPAIRS, 128, C), dtype=np.float32)
        for p in range(NPAIRS):
            for hh in range(2):
                hgl = heads[2 * p + hh]
                rows_q = 0 * C + hgl * 64 + perm
                rows_k = 1 * C + hgl * 64 + perm
                wqkT[p, :, hh * 64:(hh + 1) * 64] = w_qkv[rows_q, :].T
                wqkT[p, :, 128 + hh * 64:128 + (hh + 1) * 64] = w_qkv[rows_k, :].T
                wpTc[p, hh * 64:(hh + 1) * 64, :] = \
                    w_proj[:, hgl * 64:(hgl + 1) * 64].T
        wvTc = np.empty((C, 384), dtype=np.float32)
        for i, hgl in enumerate(heads):
            rows_v = 2 * C + hgl * 64 + np.arange(64)
            wvTc[:, i * 64:(i + 1) * 64] = w_qkv[rows_v, :].T

        in_maps.append({
            "xT": xTc.astype(BF),
            "wqkT": np.ascontiguousarray(wqkT).astype(BF),
            "wvT": wvTc.astype(BF),
            "wpT": np.ascontiguousarray(wpTc).astype(BF),
            "c2T": c2T.astype(BF),
            "s2T": s2T.astype(BF),
            "tri01": tri01.astype(BF),
            "tri256": tri256.astype(BF),
            "ident": np.eye(128, dtype=np.float32).astype(BF),
        })
    return in_maps


def _mask_is_causal(mask):
    m = mask[0, 0]
    if m.shape != (N, N):
        return False
    iu = np.triu_indices(N, k=1)
    il = np.tril_indices(N, k=0)
    return bool(np.all(m[il] == 0.0) and np.all(m[iu] <= -1e8))


def _numpy_reference(x, freqs_cos, freqs_sin, mask, w_qkv, w_proj, b_proj):
    """Exact fallback (never expected: setup_inputs' mask is causal)."""
    Bq, Nq, Cq = x.shape
    qkv = x @ w_qkv.T
    qkv = qkv.reshape(Bq, Nq, 3, H, Dh)
    q, k, v = qkv[:, :, 0], qkv[:, :, 1], qkv[:, :, 2]

    def rope(t):
        tr = t.reshape(Bq, Nq, H, Dh // 2, 2)
        a, b = tr[..., 0], tr[..., 1]
        c = freqs_cos[None, :, None, :]
        s = freqs_sin[None, :, None, :]
        return np.stack([a * c - b * s, a * s + b * c], axis=-1).reshape(t.shape)

    q, k = rope(q), rope(k)
    q = q.transpose(0, 2, 1, 3)
    k = k.transpose(0, 2, 1, 3)
    v = v.transpose(0, 2, 1, 3)
    att = np.einsum('bhqd,bhkd->bhqk', q, k) * SCALE + mask
    att = att - att.max(axis=-1, keepdims=True)
    att = np.exp(att)
    att = att / att.sum(axis=-1, keepdims=True)
    out = np.einsum('bhqk,bhkd->bhqd', att, v)
    out = out.transpose(0, 2, 1, 3).reshape(Bq, Nq, Cq)
    return (out @ w_proj.T + b_proj).astype(np.float32)


def _get_compiled():
    global _compiled
    if _compiled is None:
        _compiled = _build_program()
    return _compiled


def run_device(in_maps, trace=False, trace_kwargs=None):
    from concourse.bass_utils import run_bass_kernel_spmd
    nc = _get_compiled()
    kwargs = {}
    if trace:
        kwargs["trace"] = True
        if trace_kwargs:
            kwargs["trace_kwargs"] = trace_kwargs
    return run_bass_kernel_spmd(nc, in_maps, core_ids=list(range(NCORES)), **kwargs)


def _assemble(results, b_proj):
    y = np.empty((B, N, C), dtype=np.float32)
    for b in range(B):
        acc = results[2 * b]["ypart"].astype(np.float32).sum(axis=0)
        acc += results[2 * b + 1]["ypart"].astype(np.float32).sum(axis=0)
        y[b] = acc.T + b_proj[None, :]
    return y


def kernel(x, freqs_cos, freqs_sin, mask, w_qkv, w_proj, b_proj):
    x = np.asarray(x, dtype=np.float32)
    freqs_cos = np.asarray(freqs_cos, dtype=np.float32)
    freqs_sin = np.asarray(freqs_sin, dtype=np.float32)
    mask = np.asarray(mask, dtype=np.float32)
    w_qkv = np.asarray(w_qkv, dtype=np.float32)
    w_proj = np.asarray(w_proj, dtype=np.float32)
    b_proj = np.asarray(b_proj, dtype=np.float32)

    if x.shape != (B, N, C) or not _mask_is_causal(mask):
        return _numpy_reference(x, freqs_cos, freqs_sin, mask, w_qkv, w_proj, b_proj)

    in_maps = _host_prep(x, freqs_cos, freqs_sin, mask, w_qkv, w_proj)
    res = run_device(in_maps)
    return _assemble(res.results, b_proj)

